# revision 56
# baseline (speedup 1.0000x reference)
"""Trainium2 Bass kernel for nn_AttentionUnit (multi-head attention block), v3.

Reference math (B=2, S=2048, D=1024, H=16 heads, d_head=64, fp32):
    Q = q @ wq.T + bq ; K = k @ wk.T + bk ; V = v @ wv.T + bv
    S = QK^T / 8  (per head), causal mask + key-padding mask
    out = softmax(S) @ V  -> concat heads -> @ wo.T + bo
Sharding (8 cores): data-parallel over batch (2 groups of 4 cores),
tensor-parallel over heads (4 heads/core).  Column-parallel QKV,
row-parallel wo.

v3 changes vs v2 (156.0us):
  - Out-proj computed TRANSPOSED (out^T[dim, tok] psum tiles): the bias
    becomes a per-partition scalar so eviction is a 192ns tensor_scalar
    instead of a 658ns tensor_tensor (-15us DVE), and the partial/RS/
    output layout is [dims, tokens] (host re-transposes).
  - One ReduceScatter per q block, writing the external output directly
    (no rs_out bounce).  Block completion order is re-staged
    (qb0 -> [qb1 h0/h1] -> qb3 -> qb2 -> [qb1 h2/h3]) so the first
    three collectives drain the queue early and the tail collective
    carries a single 256KB-out block.
  - Exp split between the Act engine and the DVE by a tunable pattern;
    Act engine carries no DMAs anymore.
  - kq/v input DMAs front-loaded; score emission per-mt-gated as before.
"""

import os
import sys
from collections import deque
from contextlib import ExitStack

import numpy as np

try:
    import concourse.bass as bass
except ImportError:  # harness containers keep the repo at /opt/trn_rl_repo
    for _p in ("/opt/trn_rl_repo", "/root/.axon_site/_ro/trn_rl_repo"):
        if os.path.isdir(_p) and _p not in sys.path:
            sys.path.insert(0, _p)
    import concourse.bass as bass

from concourse import bacc

import ml_dtypes
import concourse.mybir as mybir
import concourse.tile as tile
from concourse.bass_utils import run_bass_kernel_spmd

BF16 = ml_dtypes.bfloat16

B = 2
SEQ = 2048
D = 1024
H = 16
DH = 64
NCORES = 8
G = 4            # tensor-parallel group size (cores per batch)
HPC = H // G     # heads per core
DPC = HPC * DH   # head dims per core (256)
QB = 512         # q block width
KT = 128         # k tile height
NMT = DPC // 128  # mt tiles of per-core head dims (2)
NDT = D // 128    # contraction tiles of the model dim (8)
NQB = SEQ // QB   # q blocks (4)
SUB = QB // KT    # k tiles per q block on the diagonal (4)
NOT = D // 128    # out^T dim tiles (8)
ODC = D // G      # out dims per core after RS (256)


def build_program(use_kpm=False):
    """Emit the SPMD program (identical on all 8 cores)."""
    fp32 = mybir.dt.float32
    bf16 = mybir.dt.bfloat16

    nc = bacc.Bacc(num_devices=NCORES)

    xqT = nc.declare_dram_parameter("xqT", [D, SEQ], bf16, False)
    xkT = nc.declare_dram_parameter("xkT", [D, SEQ], bf16, False)
    xvT = nc.declare_dram_parameter("xvT", [D, SEQ], bf16, False)
    wqT = nc.declare_dram_parameter("wqT", [D, DPC], bf16, False)
    wkT = nc.declare_dram_parameter("wkT", [D, DPC], bf16, False)
    wvT = nc.declare_dram_parameter("wvT", [D, DPC], bf16, False)
    wvb = nc.declare_dram_parameter("wvb", [1, DPC], bf16, False)
    woT = nc.declare_dram_parameter("woT", [DPC, D], bf16, False)
    bq2_d = nc.declare_dram_parameter("bq2", [128, NMT], fp32, False)
    bk2_d = nc.declare_dram_parameter("bk2", [128, NMT], fp32, False)
    tri_d = nc.declare_dram_parameter("tri", [KT, KT], bf16, False)
    ident_d = nc.declare_dram_parameter("ident", [128, 128], bf16, False)
    bo2_d = nc.declare_dram_parameter("bo2", [128, NOT], fp32, False)
    kpm_d = (nc.declare_dram_parameter("kpmT", [128, SEQ // 128], fp32, False)
             if use_kpm else None)
    # out^T layout: rows = [qb-block][dims-slice owned by this core],
    # cols = the block's 512 tokens.
    out_ext = nc.declare_dram_parameter("out", [NQB * ODC, QB], bf16,
                                        isOutput=True)

    partial_dram = nc.dram_tensor("partial", [NQB * D, QB], bf16)
    rs_out = nc.dram_tensor("rs_out", [NQB * ODC, QB], bf16)

    groups = [[0, 1, 2, 3], [4, 5, 6, 7]]

    with ExitStack() as ctx:
        tc = ctx.enter_context(tile.TileContext(nc, num_cores=NCORES))

        xpool = ctx.enter_context(tc.tile_pool(name="xp", bufs=28))
        persist = ctx.enter_context(tc.tile_pool(name="persist", bufs=1))
        ppool = ctx.enter_context(tc.tile_pool(name="pp", bufs=34))
        cqpool = ctx.enter_context(tc.tile_pool(name="cq", bufs=4))
        opool = ctx.enter_context(tc.tile_pool(name="op", bufs=4))
        spool = ctx.enter_context(tc.tile_pool(name="sp", bufs=8))
        psP = ctx.enter_context(tc.tile_pool(name="psP", bufs=2, space="PSUM"))
        psM = ctx.enter_context(tc.tile_pool(name="psM", bufs=2, space="PSUM"))
        psC = ctx.enter_context(tc.tile_pool(name="psC", bufs=2, space="PSUM"))

        # ---- small constants.  The ones needed in the first ~10us (exp
        # masks, K/Q biases) ride the otherwise-idle Act queue; the rest
        # (ident/bo2/wvb/wo, first used ~18us+) are DMAed later on the
        # gpsimd queue (see the "c2" sched unit) so they don't delay the
        # xq0/wq0 feed that gates the first exp. ----
        bq2_sb = persist.tile([128, NMT], fp32, tag="bq2")
        nc.scalar.dma_start(out=bq2_sb, in_=bq2_d[:, :])
        tri_sb = persist.tile([KT, KT], bf16, tag="tri")
        nc.scalar.dma_start(out=tri_sb, in_=tri_d[:, :])
        bk2_sb = persist.tile([128, NMT], fp32, tag="bk2")
        nc.gpsimd.dma_start(out=bk2_sb, in_=bk2_d[:, :])
        ones1 = persist.tile([1, 128], bf16, tag="ones1")
        nc.vector.memset(ones1, 1.0)
        # warm the Act function table (LoadActFuncSet ~1.3us) before the
        # first real exp needs it
        warm = persist.tile([1, 1], fp32, tag="warm")
        nc.vector.memset(warm, 0.0)
        nc.scalar.activation(out=warm, in_=warm,
                             func=mybir.ActivationFunctionType.Exp)
        kpm_sb = None
        if use_kpm:
            kpm_sb = persist.tile([128, SEQ // 128], fp32, tag="kpm")
            nc.scalar.dma_start(out=kpm_sb, in_=kpm_d[:, :])
        ident_sb = persist.tile([128, 128], bf16, tag="ident")
        bo2_sb = persist.tile([128, NOT], fp32, tag="bo2")
        wvb_sb = persist.tile([1, DPC], bf16, tag="wvb")
        wo_sb = [persist.tile([128, D], bf16, tag=f"wo{t}", name=f"wo{t}")
                 for t in range(NMT)]

        def consts2():
            nc.gpsimd.dma_start(out=ident_sb, in_=ident_d[:, :])
            nc.gpsimd.dma_start(out=bo2_sb, in_=bo2_d[:, :])
            nc.gpsimd.dma_start(out=wvb_sb, in_=wvb[0:1, :])
            for t in range(NMT):
                nc.gpsimd.dma_start(out=wo_sb[t],
                                    in_=woT[t * 128:(t + 1) * 128, :])

        # ---- persistent weights (consumed every block; load once) ----
        wk_t = [persist.tile([128, DPC], bf16, tag=f"wk{k}", name=f"wk{k}")
                for k in range(NDT)]
        wq_t = [persist.tile([128, DPC], bf16, tag=f"wq{k}", name=f"wq{k}")
                for k in range(NDT)]
        wv_t = [persist.tile([128, DPC], bf16, tag=f"wv{k}", name=f"wv{k}")
                for k in range(NDT)]

        # ---- persistent activations ----
        # K2/Q2 are mt-major: rows = the 128 head dims of heads (2mt, 2mt+1).
        K2 = [persist.tile([128, SEQ], bf16, tag=f"K2{t}", name=f"K2{t}")
              for t in range(NMT)]
        Q2 = [persist.tile([128, SEQ], bf16, tag=f"Q2{t}", name=f"Q2{t}")
              for t in range(NMT)]
        V_sb = [persist.tile([128, HPC, 65], bf16, tag=f"V{m}", name=f"V{m}")
                for m in range(SEQ // KT)]
        ctxT = [persist.tile([128, SEQ], bf16, tag=f"ctxT{t}", name=f"ctxT{t}")
                for t in range(NMT)]

        def kq_dma(blk):
            """Issue K/Q input DMAs for q/k columns [blk*QB, (blk+1)*QB).

            Block 0 gates the first exp, so its 32 transfers are spread
            over four queues (DVE/Act are idle at t=0) to land in ~4us
            instead of 8; later blocks ride sync/gpsimd as usual."""
            c0 = blk * QB
            xk_t, xq_t = [], []
            if blk == 0:
                e3 = [nc.sync, nc.gpsimd, nc.scalar]
                for k in range(NDT):
                    eng = e3[k % 3]
                    xk = xpool.tile([128, QB], bf16, tag="xt", name=f"xk0_{k}")
                    eng.dma_start(out=xk, in_=xkT[k * 128:(k + 1) * 128, c0:c0 + QB])
                    eng.dma_start(out=wk_t[k], in_=wkT[k * 128:(k + 1) * 128, :])
                    xq = xpool.tile([128, QB], bf16, tag="xt", name=f"xq0_{k}")
                    eng.dma_start(out=xq, in_=xqT[k * 128:(k + 1) * 128, c0:c0 + QB])
                    eng.dma_start(out=wq_t[k], in_=wqT[k * 128:(k + 1) * 128, :])
                    xk_t.append(xk)
                    xq_t.append(xq)
                return xk_t, xq_t
            eng = nc.gpsimd if blk == 1 else nc.sync
            for k in range(NDT):
                xk = xpool.tile([128, QB], bf16, tag="xt", name=f"xk{blk}_{k}")
                nc.sync.dma_start(out=xk, in_=xkT[k * 128:(k + 1) * 128, c0:c0 + QB])
                xq = xpool.tile([128, QB], bf16, tag="xt", name=f"xq{blk}_{k}")
                eng.dma_start(out=xq, in_=xqT[k * 128:(k + 1) * 128, c0:c0 + QB])
                xk_t.append(xk)
                xq_t.append(xq)
            return xk_t, xq_t

        def kq_gen(blk, mt, xk_t, xq_t):
            """K and Q projection matmuls for one mt half (yield/matmul)."""
            c0 = blk * QB
            pskq = psP.tile([128, 2 * QB], fp32, tag="p2", name="pskq")
            psk, psq = pskq[:, 0:QB], pskq[:, QB:2 * QB]
            for k in range(NDT):
                st, sp = (k == 0), (k == NDT - 1)
                nc.tensor.matmul(out=psk, rhs=xk_t[k],
                                 lhsT=wk_t[k][:, mt * 128:(mt + 1) * 128],
                                 start=st, stop=sp)
                yield
                nc.tensor.matmul(out=psq, rhs=xq_t[k],
                                 lhsT=wq_t[k][:, mt * 128:(mt + 1) * 128],
                                 start=st, stop=sp)
                yield
            nc.vector.tensor_scalar(
                out=K2[mt][:, c0:c0 + QB], in0=psk,
                scalar1=bk2_sb[:, mt:mt + 1], scalar2=None,
                op0=mybir.AluOpType.add)
            if blk == 0:
                # Act is idle before the first exp; evicting Q there
                # overlaps the serial DVE eviction pair at startup
                nc.scalar.activation(
                    out=Q2[mt][:, c0:c0 + QB], in_=psq,
                    bias=bq2_sb[:, mt:mt + 1],
                    func=mybir.ActivationFunctionType.Identity)
            else:
                nc.vector.tensor_scalar(
                    out=Q2[mt][:, c0:c0 + QB], in0=psq,
                    scalar1=bq2_sb[:, mt:mt + 1], scalar2=None,
                    op0=mybir.AluOpType.add)

        def v_dma(blk):
            c0 = blk * QB
            eng = nc.gpsimd if blk <= 1 else nc.sync
            xv_t = []
            for k in range(NDT):
                xv = xpool.tile([128, QB], bf16, tag="xt", name=f"xv{blk}_{k}")
                eng.dma_start(out=xv, in_=xvT[k * 128:(k + 1) * 128, c0:c0 + QB])
                if blk == 0:
                    eng.dma_start(out=wv_t[k], in_=wvT[k * 128:(k + 1) * 128, :])
                xv_t.append(xv)
            return xv_t

        def v_gen(blk, w, xv_t):
            """V projection half-block: tokens [blk*QB + w*256, +256) ->
            V_sb[4blk+2w], V_sb[4blk+2w+1].  Accumulates in psM quarter
            tiles (sequentially, short holds) so score pairs keep both
            psP banks for depth-2 exp during the long sc phases."""
            psv = psP.tile([128, 2 * QB], fp32, tag="p2", name="psv")
            ps = [psv[:, 0:DPC], psv[:, QB:QB + DPC]]
            for k in range(NDT):
                for i in range(2):
                    m2 = 2 * w + i
                    nc.tensor.matmul(out=ps[i], rhs=wv_t[k],
                                     lhsT=xv_t[k][:, m2 * 128:(m2 + 1) * 128],
                                     start=(k == 0), stop=False)
                    yield
            for i in range(2):
                mt = SUB * blk + 2 * w + i
                nc.tensor.matmul(out=ps[i], rhs=wvb_sb[0:1, :],
                                 lhsT=ones1[0:1, :], start=False, stop=True)
                nc.vector.tensor_copy(
                    out=V_sb[mt][:, :, 0:64],
                    in_=ps[i].rearrange("p (h e) -> p h e", h=HPC))
                nc.vector.memset(V_sb[mt][:, :, 64:65], 1.0)
                yield

        def sc_head(qb, h, pts, pump, reserve):
            """Scores + exp + causal mask for one head of q block qb."""
            q0 = qb * QB
            mt, hh = divmod(h, 2)
            krows = slice(64 * hh, 64 * hh + 64)
            nfull = SUB * qb
            for i in range(nfull // 2):
                # two full k tiles share a 2-bank PSUM tile and one wide exp
                k0 = 2 * i
                reserve("pt2")
                s2 = psP.tile([128, 2 * QB], fp32, tag="p2", name=f"s2{h}_{i}")
                pt2 = ppool.tile([128, 2 * QB], bf16, tag="pt2",
                                 name=f"pt2{h}_{i}")
                for d in range(2):
                    nc.tensor.matmul(
                        out=s2[:, d * QB:(d + 1) * QB],
                        lhsT=K2[mt][krows, (k0 + d) * KT:(k0 + d + 1) * KT],
                        rhs=Q2[mt][krows, q0:q0 + QB],
                        start=True, stop=True)
                    pump(1)
                nc.scalar.activation(
                    out=pt2, in_=s2,
                    func=mybir.ActivationFunctionType.Exp)
                if use_kpm:
                    for d in range(2):
                        nc.vector.tensor_scalar(
                            out=pt2[:, d * QB:(d + 1) * QB],
                            in0=pt2[:, d * QB:(d + 1) * QB],
                            scalar1=kpm_sb[:, k0 + d:k0 + d + 1], scalar2=None,
                            op0=mybir.AluOpType.mult)
                pts[h, k0] = pt2[:, 0:QB]
                pts[h, k0 + 1] = pt2[:, QB:2 * QB]
                pump(3)
            for kti in range(nfull, nfull + SUB):
                o = 128 * (kti - nfull)
                reserve("pt")
                s_ps = psM.tile([128, QB], fp32, tag="m", name=f"s{h}_{kti}")
                nc.tensor.matmul(
                    out=s_ps[:, o:QB],
                    lhsT=K2[mt][krows, kti * KT:(kti + 1) * KT],
                    rhs=Q2[mt][krows, q0 + o:q0 + QB],
                    start=True, stop=True)
                pt = ppool.tile([128, QB], bf16, tag="pt", name=f"pt{h}_{kti}")
                nc.scalar.activation(
                    out=pt[:, o:QB], in_=s_ps[:, o:QB],
                    func=mybir.ActivationFunctionType.Exp)
                nc.vector.tensor_mul(
                    out=pt[:, o:o + KT], in0=pt[:, o:o + KT], in1=tri_sb)
                if use_kpm:
                    nc.vector.tensor_scalar(
                        out=pt[:, o:QB], in0=pt[:, o:QB],
                        scalar1=kpm_sb[:, kti:kti + 1], scalar2=None,
                        op0=mybir.AluOpType.mult)
                pts[h, kti] = pt
                pump(3)

        def pv_gen(qb, p, pts):
            """PV for head pair p; ctx lands q-major; one transpose/chunk."""
            q0 = qb * QB
            h0, h1 = 2 * p, 2 * p + 1

            def flush(qs, cq):
                tr_ps = psC.tile([128, 128], bf16, tag="c")
                nc.tensor.transpose(out=tr_ps, in_=cq, identity=ident_sb)
                nc.vector.tensor_copy(
                    out=ctxT[p][:, q0 + qs * 128:q0 + (qs + 1) * 128],
                    in_=tr_ps)

            pend = None
            for qs in range(SUB):
                cq = cqpool.tile([128, 128], bf16, tag="cq")
                for j, h in enumerate((h0, h1)):
                    ctx_ps = psC.tile([128, 65], fp32, tag="c", name=f"ctx{j}")
                    for kti in range(SUB * qb + qs + 1):
                        nc.tensor.matmul(
                            out=ctx_ps,
                            lhsT=pts[h, kti][:, qs * 128:(qs + 1) * 128],
                            rhs=V_sb[kti][:, h, :],
                            start=(kti == 0), stop=(kti == SUB * qb + qs))
                    rcp = spool.tile([128, 1], fp32, tag="rcp")
                    nc.vector.reciprocal(out=rcp, in_=ctx_ps[:, 64:65])
                    nc.vector.tensor_scalar(
                        out=cq[:, 64 * j:64 * j + 64], in0=ctx_ps[:, 0:64],
                        scalar1=rcp, scalar2=None, op0=mybir.AluOpType.mult)
                    yield
                if pend is not None:
                    flush(*pend)
                pend = (qs, cq)
            flush(*pend)

        # partial_dram block slots in completion order (qb0, qb1, qb3,
        # qb2); each block fires its own RS as soon as its partials land.
        # assemble_output inverts PSLOT.
        PSLOT = {0: 0, 1: 1, 3: 2, 2: 3}

        def op_gen(qb):
            """Transposed out-proj for block qb -> partial -> ReduceScatter.

            psum tiles are out^T[dt*128:(dt+1)*128, 512 tokens]; the bias is
            per-partition so eviction is one tensor_scalar.  The RS scatters
            a partial slot's D rows over the 4-core group; core j receives
            dims [256j, 256j+256) directly into the external output.
            """
            q0 = qb * QB
            p0 = PSLOT[qb] * D
            for dt in range(NOT):
                ps = psM.tile([128, QB], fp32, tag="m", name="pso")
                for t in range(NMT):
                    nc.tensor.matmul(
                        out=ps,
                        lhsT=wo_sb[t][:, dt * 128:(dt + 1) * 128],
                        rhs=ctxT[t][:, q0:q0 + QB],
                        start=(t == 0), stop=(t == NMT - 1))
                    yield
                po = opool.tile([128, QB], bf16, tag="po")
                # the last block runs after all exps, so the Act engine is
                # free to take half its evictions + partial writes and
                # shorten the tail-exposed chain
                if qb == 2 and dt % 2:
                    nc.scalar.activation(
                        out=po, in_=ps, bias=bo2_sb[:, dt:dt + 1],
                        func=mybir.ActivationFunctionType.Identity)
                    nc.scalar.dma_start(
                        out=partial_dram[p0 + dt * 128:p0 + (dt + 1) * 128, :],
                        in_=po)
                else:
                    nc.vector.tensor_scalar(
                        out=po, in0=ps, scalar1=bo2_sb[:, dt:dt + 1],
                        scalar2=None, op0=mybir.AluOpType.add)
                    # partials never share the Pool queue with the RS's (a
                    # collective blocks its queue until it completes)
                    nc.sync.dma_start(
                        out=partial_dram[p0 + dt * 128:p0 + (dt + 1) * 128, :],
                        in_=po)
            s = PSLOT[qb]
            if s == 2:
                return  # slot 2 (qb3) is carried by the merged tail RS
            if s == 3:
                # qb3+qb2 finish nearly together at the exp-paced tail; one
                # merged RS beats two serialized 15us fixed costs
                nc.gpsimd.collective_compute(
                    "ReduceScatter",
                    mybir.AluOpType.add,
                    replica_groups=groups,
                    ins=[partial_dram[2 * D:4 * D, :]],
                    outs=[rs_out[2 * ODC:4 * ODC, :]],
                )
            else:
                nc.gpsimd.collective_compute(
                    "ReduceScatter",
                    mybir.AluOpType.add,
                    replica_groups=groups,
                    ins=[partial_dram[s * D:(s + 1) * D, :]],
                    outs=[rs_out[s * ODC:(s + 1) * ODC, :]],
                )

        # Emission schedule.  sc units emit inline (exp-paced, pumping
        # queued exp-free work between tiles); everything else is queued
        # and drained as filler.  Block completion order:
        # qb0 -> qb1(h0,h1) -> qb3 -> qb2 -> qb1(h2,h3), so RS[0], RS[3]
        # drain the collective queue early and the tail RS merges the two
        # late blocks.  kq projection halves (kqg blk mt) are queued
        # separately so the mt0 halves of blocks 2/3 (which gate the long
        # qb3 exp phase) drain before lower-priority filler.
        # xpool recycles 28 'xt' slots; kq/v gens must drain in DMA
        # emission order (each block's BOTH mt halves before the block
        # two later is touched) or the PE FIFO deadlocks on slot reuse.
        # Exp phases: qb0, qb1, qb2(h01) cover the PE with projection /
        # pv / op filler until all kq blocks are projected, then qb3's
        # long phase runs, then qb2(h23) closes.  Completion order is
        # qb0, qb1, qb3, qb2 with one RS each.
        sched = [
            ("kq", 0), ("c2",), ("kqg", 0, 0), ("kqg", 0, 1),
            ("sc", 0, 0), ("sc", 0, 1),
            ("kq", 1), ("kqg", 1, 0), ("kqg", 1, 1),
            ("v", 0, 0), ("v", 0, 1),
            ("sc", 0, 2), ("sc", 0, 3),
            ("kq", 2), ("kqg", 2, 0), ("kqg", 2, 1),
            ("sc", 1, 0), ("sc", 1, 1),
            ("pv", 0, 0), ("pv", 0, 1),
            ("op", 0),
            ("sc", 1, 2), ("sc", 1, 3),
            ("kq", 3),
            ("v", 1, 0), ("v", 1, 1),
            ("pv", 1, 0), ("pv", 1, 1),
            ("op", 1),
            ("kqg", 3, 0), ("kqg", 3, 1),
            ("v", 2, 0), ("v", 2, 1), ("v", 3, 0), ("v", 3, 1),
            ("sc", 2, 0), ("sc", 2, 1),
            ("pv", 2, 0),
            ("sc", 3, 0), ("sc", 3, 1), ("sc", 3, 2), ("sc", 3, 3),
            ("pv", 3, 0), ("pv", 3, 1),
            ("op", 3),
            ("sc", 2, 2), ("sc", 2, 3),
            ("pv", 2, 1),
            ("op", 2),
        ]
        pts_all = {qb: {} for qb in range(NQB)}
        queue = deque()
        kq_tiles = {}
        v_drained = set()  # (blk, w) halves fully emitted
        # live pts-tile counters: a new exp's pool slot is freed by pv
        # matmul reads of the old occupant, which must already be emitted
        # in the PE FIFO or the slot-WAR closes a dependency cycle.
        live = {"pt2": 0, "pt": 0}
        CAP = {"pt2": 31, "pt": 31}

        def v_ready(qb):
            """pv(qb, *) may only emit once v halves for kti<=4qb+3 are."""
            return all((b, w) in v_drained
                       for b in range(qb + 1) for w in range(2))

        def pump(n, light=False):
            """Drain n generator steps.  light=True prefers pv/op units
            (they hold no psP bank, so score pairs keep exp depth 2),
            scanning past queued kq/v units but stopping at the first pv
            whose V halves are not yet emitted (emission order is
            dependency order for the Tile tracker)."""
            while n > 0 and queue:
                idx = 0
                if light:
                    while idx < len(queue) and queue[idx][0][0] in ("kq", "v"):
                        idx += 1
                    if idx >= len(queue):
                        return
                    key = queue[idx][0]
                    if key[0] == "pv" and not v_ready(key[1]):
                        return
                key = queue[idx][0]
                try:
                    next(queue[idx][1])
                    n -= 1
                except StopIteration:
                    if key[0] == "v":
                        v_drained.add((key[1], key[2]))
                    elif key[0] == "pv":
                        qb = key[1]
                        live["pt2"] -= 2 * ((SUB * qb) // 2)
                        live["pt"] -= 2 * SUB
                    del queue[idx]

        def reserve(tag):
            """Before allocating a pts tile: force FIFO drains until the
            pool has a safe slot, so slot-WAR readers are always already
            in the PE FIFO."""
            while live[tag] >= CAP[tag] and queue:
                pump(4)
            live[tag] += 1

        def drain_sel(pred):
            """Drain kq units in queue order through the last one matching
            pred (slot-reuse safety: kq gens consume xt tiles in emission
            order), leaving non-kq units queued."""
            if not any(pred(key) for key, _ in queue):
                return
            last = max(i for i, (key, _) in enumerate(queue) if pred(key))
            kept = []
            for _ in range(last + 1):
                key, gen = queue.popleft()
                if key[0] == "kq":
                    for _ in gen:
                        pass
                else:
                    kept.append((key, gen))
            for item in reversed(kept):
                queue.appendleft(item)

        for unit in sched:
            kind = unit[0]
            if kind == "c2":
                consts2()
            elif kind == "kq":
                kq_tiles[unit[1]] = kq_dma(unit[1])
            elif kind == "kqg":
                blk, mt = unit[1], unit[2]
                xk_t, xq_t = kq_tiles[blk]
                queue.append((("kq", blk, mt), kq_gen(blk, mt, xk_t, xq_t)))
            elif kind == "v":
                blk, w = unit[1], unit[2]
                if w == 0:
                    kq_tiles["v", blk] = v_dma(blk)
                queue.append((unit, v_gen(blk, w, kq_tiles["v", blk])))
            elif kind == "pv":
                queue.append((unit, pv_gen(unit[1], unit[2], pts_all[unit[1]])))
            elif kind == "op":
                queue.append((unit, op_gen(unit[1])))
            else:
                qb, h = unit[1], unit[2]
                # writers this head reads must be fully emitted first
                drain_sel(lambda key: key[0] == "kq" and key[1] <= qb
                          and key[2] <= h // 2)
                sc_head(qb, h, pts_all[qb], pump, reserve)
        while queue:
            pump(1000)

        # rs_out -> out_ext bounces (collectives cannot write IO
        # tensors), emitted last so their RS-completion waits never
        # head-of-line-block earlier queue work.  Slots 0-2 are long done
        # when these run; the tail slot is split over two queues.
        for s in range(NQB - 1):
            osb = opool.tile([128, 2 * QB], bf16, tag="osb")
            nc.sync.dma_start(
                out=osb.rearrange("p (t q) -> p t q", t=2),
                in_=rs_out[s * ODC:(s + 1) * ODC, :].rearrange(
                    "(t p) q -> p t q", p=128))
            nc.sync.dma_start(
                out=out_ext[s * ODC:(s + 1) * ODC, :].rearrange(
                    "(t p) q -> p t q", p=128),
                in_=osb.rearrange("p (t q) -> p t q", t=2))
        s = NQB - 1
        for i, eng in enumerate((nc.sync, nc.scalar)):
            rows = slice(s * ODC + i * 128, s * ODC + (i + 1) * 128)
            eng.dma_start(out=out_ext[rows, :], in_=rs_out[rows, :])

    nc.compile()
    return nc


def prep_core_inputs(inputs, core):
    """Host-side shard/layout prep for one core.  Pure layout + dtype work."""
    b, g = divmod(core, G)
    sl = slice(g * DPC, (g + 1) * DPC)
    s = 1.0 / np.sqrt(DH)

    def xT(x):
        return np.ascontiguousarray(np.asarray(x)[b].T).astype(BF16)

    def wT(w, scale=1.0):
        wt = np.asarray(w)[sl, :].T.astype(np.float32)
        return (wt * scale).astype(BF16)

    def b2(bias, scale=1.0):
        bb = (np.asarray(bias)[sl].astype(np.float32) * scale)
        return np.ascontiguousarray(bb.reshape(NMT, 128).T)

    kpm = np.asarray(inputs["key_padding_mask"])
    tri = (np.arange(KT)[:, None] <= np.arange(KT)[None, :]).astype(BF16)

    d = {
        "xqT": xT(inputs["q_input"]),
        "xkT": xT(inputs["k_input"]),
        "xvT": xT(inputs["v_input"]),
        "wqT": wT(inputs["wq"], s),
        "wkT": wT(inputs["wk"]),
        "wvT": wT(inputs["wv"]),
        "wvb": np.asarray(inputs["bv"])[sl].astype(BF16)[None, :],
        "woT": np.ascontiguousarray(np.asarray(inputs["wo"]).T[sl, :]).astype(BF16),
        "bq2": b2(inputs["bq"], s),
        "bk2": b2(inputs["bk"]),
        "tri": tri,
        "ident": np.eye(128, dtype=BF16),
        # out^T bias: per-partition scalars, one column per 128-dim tile
        "bo2": np.ascontiguousarray(
            np.asarray(inputs["bo"]).astype(np.float32).reshape(NOT, 128).T / G),
    }
    if kpm.any():
        # kpmT[p, kti] = 0.0 for padded key (128*kti + p), else 1.0
        d["kpmT"] = np.ascontiguousarray(
            1.0 - kpm[b].astype(np.float32).reshape(SEQ // 128, 128).T)
    return d


SLOT_QB = [0, 1, 3, 2]  # partial/output slot -> q block (inverse of PSLOT)


def assemble_output(core_outs):
    """core 4b+j returns out^T chunks.  Slots 0/1 come from single-block
    RS's: rows [s*256, s*256+256) hold dims [256j, 256j+256) of block
    SLOT_QB[s].  Rows [512:1024) come from the merged slot-2+3 RS, which
    scatters 512 contiguous rows per core: core j holds dims
    [(j%2)*512, ..+512) of block SLOT_QB[2 + j//2]."""
    out = np.empty((B, SEQ, D), dtype=np.float32)
    for core in range(NCORES):
        b, j = divmod(core, G)
        co = np.asarray(core_outs[core]).astype(np.float32)
        for s in range(2):
            qb = SLOT_QB[s]
            blockT = co[s * ODC:(s + 1) * ODC, :]  # [256 dims, 512 tok]
            out[b, qb * QB:(qb + 1) * QB, ODC * j:ODC * (j + 1)] = blockT.T
        qb = SLOT_QB[2 + j // 2]
        d0 = (j % 2) * 512
        blockT = co[2 * ODC:4 * ODC, :]  # [512 dims, 512 tok]
        out[b, qb * QB:(qb + 1) * QB, d0:d0 + 512] = blockT.T
    return out


_CACHED = {}


def _get_nc(use_kpm=False):
    if use_kpm not in _CACHED:
        _CACHED[use_kpm] = build_program(use_kpm=use_kpm)
    return _CACHED[use_kpm]


def kernel(**inputs) -> np.ndarray:
    use_kpm = bool(np.asarray(inputs["key_padding_mask"]).any())
    nc = _get_nc(use_kpm)
    in_maps = [prep_core_inputs(inputs, core) for core in range(NCORES)]
    res = run_bass_kernel_spmd(nc, in_maps, core_ids=list(range(NCORES)))
    return assemble_output([res.results[c]["out"] for c in range(NCORES)])


if __name__ == "__main__":
    nc = build_program()
    print("program built ok")


# revision 58
# speedup vs baseline: 1.0119x; 1.0119x over previous
"""Trainium2 Bass kernel for nn_AttentionUnit (multi-head attention block), v3.

Reference math (B=2, S=2048, D=1024, H=16 heads, d_head=64, fp32):
    Q = q @ wq.T + bq ; K = k @ wk.T + bk ; V = v @ wv.T + bv
    S = QK^T / 8  (per head), causal mask + key-padding mask
    out = softmax(S) @ V  -> concat heads -> @ wo.T + bo
Sharding (8 cores): data-parallel over batch (2 groups of 4 cores),
tensor-parallel over heads (4 heads/core).  Column-parallel QKV,
row-parallel wo.

v3 changes vs v2 (156.0us):
  - Out-proj computed TRANSPOSED (out^T[dim, tok] psum tiles): the bias
    becomes a per-partition scalar so eviction is a 192ns tensor_scalar
    instead of a 658ns tensor_tensor (-15us DVE), and the partial/RS/
    output layout is [dims, tokens] (host re-transposes).
  - One ReduceScatter per q block, writing the external output directly
    (no rs_out bounce).  Block completion order is re-staged
    (qb0 -> [qb1 h0/h1] -> qb3 -> qb2 -> [qb1 h2/h3]) so the first
    three collectives drain the queue early and the tail collective
    carries a single 256KB-out block.
  - Exp split between the Act engine and the DVE by a tunable pattern;
    Act engine carries no DMAs anymore.
  - kq/v input DMAs front-loaded; score emission per-mt-gated as before.
"""

import os
import sys
from collections import deque
from contextlib import ExitStack

import numpy as np

try:
    import concourse.bass as bass
except ImportError:  # harness containers keep the repo at /opt/trn_rl_repo
    for _p in ("/opt/trn_rl_repo", "/root/.axon_site/_ro/trn_rl_repo"):
        if os.path.isdir(_p) and _p not in sys.path:
            sys.path.insert(0, _p)
    import concourse.bass as bass

from concourse import bacc

import ml_dtypes
import concourse.mybir as mybir
import concourse.tile as tile
from concourse.bass_utils import run_bass_kernel_spmd

BF16 = ml_dtypes.bfloat16

B = 2
SEQ = 2048
D = 1024
H = 16
DH = 64
NCORES = 8
G = 4            # tensor-parallel group size (cores per batch)
HPC = H // G     # heads per core
DPC = HPC * DH   # head dims per core (256)
QB = 512         # q block width
KT = 128         # k tile height
NMT = DPC // 128  # mt tiles of per-core head dims (2)
NDT = D // 128    # contraction tiles of the model dim (8)
NQB = SEQ // QB   # q blocks (4)
SUB = QB // KT    # k tiles per q block on the diagonal (4)
NOT = D // 128    # out^T dim tiles (8)
ODC = D // G      # out dims per core after RS (256)


def build_program(use_kpm=False):
    """Emit the SPMD program (identical on all 8 cores)."""
    fp32 = mybir.dt.float32
    bf16 = mybir.dt.bfloat16

    nc = bacc.Bacc(num_devices=NCORES)

    xqT = nc.declare_dram_parameter("xqT", [D, SEQ], bf16, False)
    xkT = nc.declare_dram_parameter("xkT", [D, SEQ], bf16, False)
    xvT = nc.declare_dram_parameter("xvT", [D, SEQ], bf16, False)
    wqT = nc.declare_dram_parameter("wqT", [D, DPC], bf16, False)
    wkT = nc.declare_dram_parameter("wkT", [D, DPC], bf16, False)
    wvT = nc.declare_dram_parameter("wvT", [D, DPC], bf16, False)
    wvb = nc.declare_dram_parameter("wvb", [1, DPC], bf16, False)
    woT = nc.declare_dram_parameter("woT", [DPC, D], bf16, False)
    bq2_d = nc.declare_dram_parameter("bq2", [128, NMT], fp32, False)
    bk2_d = nc.declare_dram_parameter("bk2", [128, NMT], fp32, False)
    tri_d = nc.declare_dram_parameter("tri", [KT, KT], bf16, False)
    ident_d = nc.declare_dram_parameter("ident", [128, 128], bf16, False)
    bo2_d = nc.declare_dram_parameter("bo2", [128, NOT], fp32, False)
    kpm_d = (nc.declare_dram_parameter("kpmT", [128, SEQ // 128], fp32, False)
             if use_kpm else None)
    # out^T layout: rows = [qb-block][dims-slice owned by this core],
    # cols = the block's 512 tokens.
    out_ext = nc.declare_dram_parameter("out", [NQB * ODC, QB], bf16,
                                        isOutput=True)

    partial_dram = nc.dram_tensor("partial", [NQB * D, QB], bf16)
    rs_out = nc.dram_tensor("rs_out", [NQB * ODC, QB], bf16)

    groups = [[0, 1, 2, 3], [4, 5, 6, 7]]

    with ExitStack() as ctx:
        tc = ctx.enter_context(tile.TileContext(nc, num_cores=NCORES))

        xpool = ctx.enter_context(tc.tile_pool(name="xp", bufs=28))
        persist = ctx.enter_context(tc.tile_pool(name="persist", bufs=1))
        ppool = ctx.enter_context(tc.tile_pool(name="pp", bufs=34))
        cqpool = ctx.enter_context(tc.tile_pool(name="cq", bufs=4))
        opool = ctx.enter_context(tc.tile_pool(name="op", bufs=4))
        spool = ctx.enter_context(tc.tile_pool(name="sp", bufs=8))
        psP = ctx.enter_context(tc.tile_pool(name="psP", bufs=2, space="PSUM"))
        psM = ctx.enter_context(tc.tile_pool(name="psM", bufs=2, space="PSUM"))
        psC = ctx.enter_context(tc.tile_pool(name="psC", bufs=2, space="PSUM"))

        # ---- small constants.  The ones needed in the first ~10us (exp
        # masks, K/Q biases) ride the otherwise-idle Act queue; the rest
        # (ident/bo2/wvb/wo, first used ~18us+) are DMAed later on the
        # gpsimd queue (see the "c2" sched unit) so they don't delay the
        # xq0/wq0 feed that gates the first exp. ----
        bq2_sb = persist.tile([128, NMT], fp32, tag="bq2")
        nc.scalar.dma_start(out=bq2_sb, in_=bq2_d[:, :])
        tri_sb = persist.tile([KT, KT], bf16, tag="tri")
        nc.scalar.dma_start(out=tri_sb, in_=tri_d[:, :])
        bk2_sb = persist.tile([128, NMT], fp32, tag="bk2")
        nc.gpsimd.dma_start(out=bk2_sb, in_=bk2_d[:, :])
        ones1 = persist.tile([1, 128], bf16, tag="ones1")
        nc.vector.memset(ones1, 1.0)
        # warm the Act function table (LoadActFuncSet ~1.3us) before the
        # first real exp needs it
        warm = persist.tile([1, 1], fp32, tag="warm")
        nc.vector.memset(warm, 0.0)
        nc.scalar.activation(out=warm, in_=warm,
                             func=mybir.ActivationFunctionType.Exp)
        kpm_sb = None
        if use_kpm:
            kpm_sb = persist.tile([128, SEQ // 128], fp32, tag="kpm")
            nc.scalar.dma_start(out=kpm_sb, in_=kpm_d[:, :])
        ident_sb = persist.tile([128, 128], bf16, tag="ident")
        bo2_sb = persist.tile([128, NOT], fp32, tag="bo2")
        wvb_sb = persist.tile([1, DPC], bf16, tag="wvb")
        wo_sb = [persist.tile([128, D], bf16, tag=f"wo{t}", name=f"wo{t}")
                 for t in range(NMT)]

        def consts2():
            nc.gpsimd.dma_start(out=ident_sb, in_=ident_d[:, :])
            nc.gpsimd.dma_start(out=bo2_sb, in_=bo2_d[:, :])
            nc.gpsimd.dma_start(out=wvb_sb, in_=wvb[0:1, :])
            for t in range(NMT):
                nc.gpsimd.dma_start(out=wo_sb[t],
                                    in_=woT[t * 128:(t + 1) * 128, :])

        # ---- persistent weights (consumed every block; load once) ----
        wk_t = [persist.tile([128, DPC], bf16, tag=f"wk{k}", name=f"wk{k}")
                for k in range(NDT)]
        wq_t = [persist.tile([128, DPC], bf16, tag=f"wq{k}", name=f"wq{k}")
                for k in range(NDT)]
        wv_t = [persist.tile([128, DPC], bf16, tag=f"wv{k}", name=f"wv{k}")
                for k in range(NDT)]

        # ---- persistent activations ----
        # K2/Q2 are mt-major: rows = the 128 head dims of heads (2mt, 2mt+1).
        K2 = [persist.tile([128, SEQ], bf16, tag=f"K2{t}", name=f"K2{t}")
              for t in range(NMT)]
        Q2 = [persist.tile([128, SEQ], bf16, tag=f"Q2{t}", name=f"Q2{t}")
              for t in range(NMT)]
        V_sb = [persist.tile([128, HPC, 65], bf16, tag=f"V{m}", name=f"V{m}")
                for m in range(SEQ // KT)]
        ctxT = [persist.tile([128, SEQ], bf16, tag=f"ctxT{t}", name=f"ctxT{t}")
                for t in range(NMT)]

        def kq_dma(blk):
            """Issue K/Q input DMAs for q/k columns [blk*QB, (blk+1)*QB).

            Block 0 gates the first exp, so its 32 transfers are spread
            over four queues (DVE/Act are idle at t=0) to land in ~4us
            instead of 8; later blocks ride sync/gpsimd as usual."""
            c0 = blk * QB
            xk_t, xq_t = [], []
            if blk == 0:
                e3 = [nc.sync, nc.gpsimd, nc.scalar]
                for k in range(NDT):
                    eng = e3[k % 3]
                    xk = xpool.tile([128, QB], bf16, tag="xt", name=f"xk0_{k}")
                    eng.dma_start(out=xk, in_=xkT[k * 128:(k + 1) * 128, c0:c0 + QB])
                    eng.dma_start(out=wk_t[k], in_=wkT[k * 128:(k + 1) * 128, :])
                    xq = xpool.tile([128, QB], bf16, tag="xt", name=f"xq0_{k}")
                    eng.dma_start(out=xq, in_=xqT[k * 128:(k + 1) * 128, c0:c0 + QB])
                    eng.dma_start(out=wq_t[k], in_=wqT[k * 128:(k + 1) * 128, :])
                    xk_t.append(xk)
                    xq_t.append(xq)
                return xk_t, xq_t
            eng = nc.gpsimd if blk == 1 else nc.sync
            for k in range(NDT):
                xk = xpool.tile([128, QB], bf16, tag="xt", name=f"xk{blk}_{k}")
                nc.sync.dma_start(out=xk, in_=xkT[k * 128:(k + 1) * 128, c0:c0 + QB])
                xq = xpool.tile([128, QB], bf16, tag="xt", name=f"xq{blk}_{k}")
                eng.dma_start(out=xq, in_=xqT[k * 128:(k + 1) * 128, c0:c0 + QB])
                xk_t.append(xk)
                xq_t.append(xq)
            return xk_t, xq_t

        def kq_gen(blk, mt, xk_t, xq_t):
            """K and Q projection matmuls for one mt half (yield/matmul)."""
            c0 = blk * QB
            pskq = psP.tile([128, 2 * QB], fp32, tag="p2", name="pskq")
            psk, psq = pskq[:, 0:QB], pskq[:, QB:2 * QB]
            for k in range(NDT):
                st, sp = (k == 0), (k == NDT - 1)
                nc.tensor.matmul(out=psk, rhs=xk_t[k],
                                 lhsT=wk_t[k][:, mt * 128:(mt + 1) * 128],
                                 start=st, stop=sp)
                yield
                nc.tensor.matmul(out=psq, rhs=xq_t[k],
                                 lhsT=wq_t[k][:, mt * 128:(mt + 1) * 128],
                                 start=st, stop=sp)
                yield
            nc.vector.tensor_scalar(
                out=K2[mt][:, c0:c0 + QB], in0=psk,
                scalar1=bk2_sb[:, mt:mt + 1], scalar2=None,
                op0=mybir.AluOpType.add)
            if blk == 0:
                # Act is idle before the first exp; evicting Q there
                # overlaps the serial DVE eviction pair at startup
                nc.scalar.activation(
                    out=Q2[mt][:, c0:c0 + QB], in_=psq,
                    bias=bq2_sb[:, mt:mt + 1],
                    func=mybir.ActivationFunctionType.Identity)
            else:
                nc.vector.tensor_scalar(
                    out=Q2[mt][:, c0:c0 + QB], in0=psq,
                    scalar1=bq2_sb[:, mt:mt + 1], scalar2=None,
                    op0=mybir.AluOpType.add)

        def v_dma(blk):
            c0 = blk * QB
            eng = nc.gpsimd if blk <= 1 else nc.sync
            xv_t = []
            for k in range(NDT):
                xv = xpool.tile([128, QB], bf16, tag="xt", name=f"xv{blk}_{k}")
                eng.dma_start(out=xv, in_=xvT[k * 128:(k + 1) * 128, c0:c0 + QB])
                if blk == 0:
                    eng.dma_start(out=wv_t[k], in_=wvT[k * 128:(k + 1) * 128, :])
                xv_t.append(xv)
            return xv_t

        def v_gen(blk, w, xv_t):
            """V projection half-block: tokens [blk*QB + w*256, +256) ->
            V_sb[4blk+2w], V_sb[4blk+2w+1].  Accumulates in psM quarter
            tiles (sequentially, short holds) so score pairs keep both
            psP banks for depth-2 exp during the long sc phases."""
            psv = psP.tile([128, 2 * QB], fp32, tag="p2", name="psv")
            ps = [psv[:, 0:DPC], psv[:, QB:QB + DPC]]
            for k in range(NDT):
                for i in range(2):
                    m2 = 2 * w + i
                    nc.tensor.matmul(out=ps[i], rhs=wv_t[k],
                                     lhsT=xv_t[k][:, m2 * 128:(m2 + 1) * 128],
                                     start=(k == 0), stop=False)
                    yield
            for i in range(2):
                mt = SUB * blk + 2 * w + i
                nc.tensor.matmul(out=ps[i], rhs=wvb_sb[0:1, :],
                                 lhsT=ones1[0:1, :], start=False, stop=True)
                nc.vector.tensor_copy(
                    out=V_sb[mt][:, :, 0:64],
                    in_=ps[i].rearrange("p (h e) -> p h e", h=HPC))
                nc.vector.memset(V_sb[mt][:, :, 64:65], 1.0)
                yield

        def sc_head(qb, h, pts, pump, reserve):
            """Scores + exp + causal mask for one head of q block qb."""
            q0 = qb * QB
            mt, hh = divmod(h, 2)
            krows = slice(64 * hh, 64 * hh + 64)
            nfull = SUB * qb
            for i in range(nfull // 2):
                # two full k tiles share a 2-bank PSUM tile and one wide exp
                k0 = 2 * i
                reserve("pt2")
                s2 = psP.tile([128, 2 * QB], fp32, tag="p2", name=f"s2{h}_{i}")
                pt2 = ppool.tile([128, 2 * QB], bf16, tag="pt2",
                                 name=f"pt2{h}_{i}")
                for d in range(2):
                    nc.tensor.matmul(
                        out=s2[:, d * QB:(d + 1) * QB],
                        lhsT=K2[mt][krows, (k0 + d) * KT:(k0 + d + 1) * KT],
                        rhs=Q2[mt][krows, q0:q0 + QB],
                        start=True, stop=True)
                    pump(1)
                nc.scalar.activation(
                    out=pt2, in_=s2,
                    func=mybir.ActivationFunctionType.Exp)
                if use_kpm:
                    for d in range(2):
                        nc.vector.tensor_scalar(
                            out=pt2[:, d * QB:(d + 1) * QB],
                            in0=pt2[:, d * QB:(d + 1) * QB],
                            scalar1=kpm_sb[:, k0 + d:k0 + d + 1], scalar2=None,
                            op0=mybir.AluOpType.mult)
                pts[h, k0] = pt2[:, 0:QB]
                pts[h, k0 + 1] = pt2[:, QB:2 * QB]
                pump(3)
            for kti in range(nfull, nfull + SUB):
                o = 128 * (kti - nfull)
                reserve("pt")
                s_ps = psM.tile([128, QB], fp32, tag="m", name=f"s{h}_{kti}")
                nc.tensor.matmul(
                    out=s_ps[:, o:QB],
                    lhsT=K2[mt][krows, kti * KT:(kti + 1) * KT],
                    rhs=Q2[mt][krows, q0 + o:q0 + QB],
                    start=True, stop=True)
                pt = ppool.tile([128, QB], bf16, tag="pt", name=f"pt{h}_{kti}")
                nc.scalar.activation(
                    out=pt[:, o:QB], in_=s_ps[:, o:QB],
                    func=mybir.ActivationFunctionType.Exp)
                nc.vector.tensor_mul(
                    out=pt[:, o:o + KT], in0=pt[:, o:o + KT], in1=tri_sb)
                if use_kpm:
                    nc.vector.tensor_scalar(
                        out=pt[:, o:QB], in0=pt[:, o:QB],
                        scalar1=kpm_sb[:, kti:kti + 1], scalar2=None,
                        op0=mybir.AluOpType.mult)
                pts[h, kti] = pt
                pump(3)

        def pv_gen(qb, p, pts):
            """PV for head pair p; ctx lands q-major; one transpose/chunk."""
            q0 = qb * QB
            h0, h1 = 2 * p, 2 * p + 1

            def flush(qs, cq):
                tr_ps = psC.tile([128, 128], bf16, tag="c")
                nc.tensor.transpose(out=tr_ps, in_=cq, identity=ident_sb)
                nc.vector.tensor_copy(
                    out=ctxT[p][:, q0 + qs * 128:q0 + (qs + 1) * 128],
                    in_=tr_ps)

            pend = None
            for qs in range(SUB):
                cq = cqpool.tile([128, 128], bf16, tag="cq")
                for j, h in enumerate((h0, h1)):
                    ctx_ps = psC.tile([128, 65], fp32, tag="c", name=f"ctx{j}")
                    for kti in range(SUB * qb + qs + 1):
                        nc.tensor.matmul(
                            out=ctx_ps,
                            lhsT=pts[h, kti][:, qs * 128:(qs + 1) * 128],
                            rhs=V_sb[kti][:, h, :],
                            start=(kti == 0), stop=(kti == SUB * qb + qs))
                        if kti % 5 == 4:
                            yield
                    rcp = spool.tile([128, 1], fp32, tag="rcp")
                    nc.vector.reciprocal(out=rcp, in_=ctx_ps[:, 64:65])
                    nc.vector.tensor_scalar(
                        out=cq[:, 64 * j:64 * j + 64], in0=ctx_ps[:, 0:64],
                        scalar1=rcp, scalar2=None, op0=mybir.AluOpType.mult)
                    yield
                if pend is not None:
                    flush(*pend)
                pend = (qs, cq)
            flush(*pend)

        # partial_dram block slots in completion order (qb0, qb1, qb3,
        # qb2); each block fires its own RS as soon as its partials land.
        # assemble_output inverts PSLOT.
        PSLOT = {0: 0, 1: 1, 3: 2, 2: 3}

        def op_gen(qb):
            """Transposed out-proj for block qb -> partial -> ReduceScatter.

            psum tiles are out^T[dt*128:(dt+1)*128, 512 tokens]; the bias is
            per-partition so eviction is one tensor_scalar.  The RS scatters
            a partial slot's D rows over the 4-core group; core j receives
            dims [256j, 256j+256) directly into the external output.
            """
            q0 = qb * QB
            p0 = PSLOT[qb] * D
            for dt in range(NOT):
                ps = psM.tile([128, QB], fp32, tag="m", name="pso")
                for t in range(NMT):
                    nc.tensor.matmul(
                        out=ps,
                        lhsT=wo_sb[t][:, dt * 128:(dt + 1) * 128],
                        rhs=ctxT[t][:, q0:q0 + QB],
                        start=(t == 0), stop=(t == NMT - 1))
                    yield
                po = opool.tile([128, QB], bf16, tag="po")
                # the last block runs after all exps, so the Act engine is
                # free to take half its evictions + partial writes and
                # shorten the tail-exposed chain
                if qb == 2 and dt % 2:
                    nc.scalar.activation(
                        out=po, in_=ps, bias=bo2_sb[:, dt:dt + 1],
                        func=mybir.ActivationFunctionType.Identity)
                    nc.scalar.dma_start(
                        out=partial_dram[p0 + dt * 128:p0 + (dt + 1) * 128, :],
                        in_=po)
                else:
                    nc.vector.tensor_scalar(
                        out=po, in0=ps, scalar1=bo2_sb[:, dt:dt + 1],
                        scalar2=None, op0=mybir.AluOpType.add)
                    # partials never share the Pool queue with the RS's (a
                    # collective blocks its queue until it completes)
                    nc.sync.dma_start(
                        out=partial_dram[p0 + dt * 128:p0 + (dt + 1) * 128, :],
                        in_=po)
            s = PSLOT[qb]
            if s == 2:
                return  # slot 2 (qb3) is carried by the merged tail RS
            if s == 3:
                # qb3+qb2 finish nearly together at the exp-paced tail; one
                # merged RS beats two serialized 15us fixed costs
                nc.gpsimd.collective_compute(
                    "ReduceScatter",
                    mybir.AluOpType.add,
                    replica_groups=groups,
                    ins=[partial_dram[2 * D:4 * D, :]],
                    outs=[rs_out[2 * ODC:4 * ODC, :]],
                )
            else:
                nc.gpsimd.collective_compute(
                    "ReduceScatter",
                    mybir.AluOpType.add,
                    replica_groups=groups,
                    ins=[partial_dram[s * D:(s + 1) * D, :]],
                    outs=[rs_out[s * ODC:(s + 1) * ODC, :]],
                )

        # Emission schedule.  sc units emit inline (exp-paced, pumping
        # queued exp-free work between tiles); everything else is queued
        # and drained as filler.  Block completion order:
        # qb0 -> qb1(h0,h1) -> qb3 -> qb2 -> qb1(h2,h3), so RS[0], RS[3]
        # drain the collective queue early and the tail RS merges the two
        # late blocks.  kq projection halves (kqg blk mt) are queued
        # separately so the mt0 halves of blocks 2/3 (which gate the long
        # qb3 exp phase) drain before lower-priority filler.
        # xpool recycles 28 'xt' slots; kq/v gens must drain in DMA
        # emission order (each block's BOTH mt halves before the block
        # two later is touched) or the PE FIFO deadlocks on slot reuse.
        # Exp phases: qb0, qb1, qb2(h01) cover the PE with projection /
        # pv / op filler until all kq blocks are projected, then qb3's
        # long phase runs, then qb2(h23) closes.  Completion order is
        # qb0, qb1, qb3, qb2 with one RS each.
        sched = [
            ("kq", 0), ("c2",), ("kqg", 0, 0), ("kqg", 0, 1),
            ("sc", 0, 0), ("sc", 0, 1),
            ("kq", 1), ("kqg", 1, 0), ("kqg", 1, 1),
            ("v", 0, 0), ("v", 0, 1),
            ("sc", 0, 2), ("sc", 0, 3),
            ("kq", 2), ("kqg", 2, 0), ("kqg", 2, 1),
            ("sc", 1, 0), ("sc", 1, 1),
            ("pv", 0, 0), ("pv", 0, 1),
            ("op", 0),
            ("sc", 1, 2), ("sc", 1, 3),
            ("kq", 3),
            ("v", 1, 0), ("v", 1, 1),
            ("pv", 1, 0), ("pv", 1, 1),
            ("op", 1),
            ("kqg", 3, 0), ("kqg", 3, 1),
            ("v", 2, 0), ("v", 2, 1), ("v", 3, 0), ("v", 3, 1),
            ("sc", 2, 0), ("sc", 2, 1),
            ("pv", 2, 0),
            ("sc", 3, 0), ("sc", 3, 1), ("sc", 3, 2), ("sc", 3, 3),
            ("pv", 3, 0), ("pv", 3, 1),
            ("op", 3),
            ("sc", 2, 2), ("sc", 2, 3),
            ("pv", 2, 1),
            ("op", 2),
        ]
        pts_all = {qb: {} for qb in range(NQB)}
        queue = deque()
        kq_tiles = {}
        v_drained = set()  # (blk, w) halves fully emitted
        # live pts-tile counters: a new exp's pool slot is freed by pv
        # matmul reads of the old occupant, which must already be emitted
        # in the PE FIFO or the slot-WAR closes a dependency cycle.
        live = {"pt2": 0, "pt": 0}
        CAP = {"pt2": 31, "pt": 31}

        def v_ready(qb):
            """pv(qb, *) may only emit once v halves for kti<=4qb+3 are."""
            return all((b, w) in v_drained
                       for b in range(qb + 1) for w in range(2))

        def pump(n, light=False):
            """Drain n generator steps.  light=True prefers pv/op units
            (they hold no psP bank, so score pairs keep exp depth 2),
            scanning past queued kq/v units but stopping at the first pv
            whose V halves are not yet emitted (emission order is
            dependency order for the Tile tracker)."""
            while n > 0 and queue:
                idx = 0
                if light:
                    while idx < len(queue) and queue[idx][0][0] in ("kq", "v"):
                        idx += 1
                    if idx >= len(queue):
                        return
                    key = queue[idx][0]
                    if key[0] == "pv" and not v_ready(key[1]):
                        return
                key = queue[idx][0]
                try:
                    next(queue[idx][1])
                    n -= 1
                except StopIteration:
                    if key[0] == "v":
                        v_drained.add((key[1], key[2]))
                    elif key[0] == "pv":
                        qb = key[1]
                        live["pt2"] -= 2 * ((SUB * qb) // 2)
                        live["pt"] -= 2 * SUB
                    del queue[idx]

        def reserve(tag):
            """Before allocating a pts tile: force FIFO drains until the
            pool has a safe slot, so slot-WAR readers are always already
            in the PE FIFO."""
            while live[tag] >= CAP[tag] and queue:
                pump(4)
            live[tag] += 1

        def drain_sel(pred):
            """Drain kq units in queue order through the last one matching
            pred (slot-reuse safety: kq gens consume xt tiles in emission
            order), leaving non-kq units queued."""
            if not any(pred(key) for key, _ in queue):
                return
            last = max(i for i, (key, _) in enumerate(queue) if pred(key))
            kept = []
            for _ in range(last + 1):
                key, gen = queue.popleft()
                if key[0] == "kq":
                    for _ in gen:
                        pass
                else:
                    kept.append((key, gen))
            for item in reversed(kept):
                queue.appendleft(item)

        for unit in sched:
            kind = unit[0]
            if kind == "c2":
                consts2()
            elif kind == "kq":
                kq_tiles[unit[1]] = kq_dma(unit[1])
            elif kind == "kqg":
                blk, mt = unit[1], unit[2]
                xk_t, xq_t = kq_tiles[blk]
                queue.append((("kq", blk, mt), kq_gen(blk, mt, xk_t, xq_t)))
            elif kind == "v":
                blk, w = unit[1], unit[2]
                if w == 0:
                    kq_tiles["v", blk] = v_dma(blk)
                queue.append((unit, v_gen(blk, w, kq_tiles["v", blk])))
            elif kind == "pv":
                queue.append((unit, pv_gen(unit[1], unit[2], pts_all[unit[1]])))
            elif kind == "op":
                queue.append((unit, op_gen(unit[1])))
            else:
                qb, h = unit[1], unit[2]
                # writers this head reads must be fully emitted first
                drain_sel(lambda key: key[0] == "kq" and key[1] <= qb
                          and key[2] <= h // 2)
                sc_head(qb, h, pts_all[qb], pump, reserve)
        while queue:
            pump(1000)

        # rs_out -> out_ext bounces (collectives cannot write IO
        # tensors), emitted last so their RS-completion waits never
        # head-of-line-block earlier queue work.  Slots 0-2 are long done
        # when these run; the tail slot is split over two queues.
        for s in range(NQB - 1):
            osb = opool.tile([128, 2 * QB], bf16, tag="osb")
            nc.sync.dma_start(
                out=osb.rearrange("p (t q) -> p t q", t=2),
                in_=rs_out[s * ODC:(s + 1) * ODC, :].rearrange(
                    "(t p) q -> p t q", p=128))
            nc.sync.dma_start(
                out=out_ext[s * ODC:(s + 1) * ODC, :].rearrange(
                    "(t p) q -> p t q", p=128),
                in_=osb.rearrange("p (t q) -> p t q", t=2))
        s = NQB - 1
        for i, eng in enumerate((nc.sync, nc.scalar)):
            rows = slice(s * ODC + i * 128, s * ODC + (i + 1) * 128)
            osb = opool.tile([128, QB], bf16, tag="osb2")
            eng.dma_start(out=osb, in_=rs_out[rows, :])
            eng.dma_start(out=out_ext[rows, :], in_=osb)

    nc.compile()
    return nc


def prep_core_inputs(inputs, core):
    """Host-side shard/layout prep for one core.  Pure layout + dtype work."""
    b, g = divmod(core, G)
    sl = slice(g * DPC, (g + 1) * DPC)
    s = 1.0 / np.sqrt(DH)

    def xT(x):
        return np.ascontiguousarray(np.asarray(x)[b].T).astype(BF16)

    def wT(w, scale=1.0):
        wt = np.asarray(w)[sl, :].T.astype(np.float32)
        return (wt * scale).astype(BF16)

    def b2(bias, scale=1.0):
        bb = (np.asarray(bias)[sl].astype(np.float32) * scale)
        return np.ascontiguousarray(bb.reshape(NMT, 128).T)

    kpm = np.asarray(inputs["key_padding_mask"])
    tri = (np.arange(KT)[:, None] <= np.arange(KT)[None, :]).astype(BF16)

    d = {
        "xqT": xT(inputs["q_input"]),
        "xkT": xT(inputs["k_input"]),
        "xvT": xT(inputs["v_input"]),
        "wqT": wT(inputs["wq"], s),
        "wkT": wT(inputs["wk"]),
        "wvT": wT(inputs["wv"]),
        "wvb": np.asarray(inputs["bv"])[sl].astype(BF16)[None, :],
        "woT": np.ascontiguousarray(np.asarray(inputs["wo"]).T[sl, :]).astype(BF16),
        "bq2": b2(inputs["bq"], s),
        "bk2": b2(inputs["bk"]),
        "tri": tri,
        "ident": np.eye(128, dtype=BF16),
        # out^T bias: per-partition scalars, one column per 128-dim tile
        "bo2": np.ascontiguousarray(
            np.asarray(inputs["bo"]).astype(np.float32).reshape(NOT, 128).T / G),
    }
    if kpm.any():
        # kpmT[p, kti] = 0.0 for padded key (128*kti + p), else 1.0
        d["kpmT"] = np.ascontiguousarray(
            1.0 - kpm[b].astype(np.float32).reshape(SEQ // 128, 128).T)
    return d


SLOT_QB = [0, 1, 3, 2]  # partial/output slot -> q block (inverse of PSLOT)


def assemble_output(core_outs):
    """core 4b+j returns out^T chunks.  Slots 0/1 come from single-block
    RS's: rows [s*256, s*256+256) hold dims [256j, 256j+256) of block
    SLOT_QB[s].  Rows [512:1024) come from the merged slot-2+3 RS, which
    scatters 512 contiguous rows per core: core j holds dims
    [(j%2)*512, ..+512) of block SLOT_QB[2 + j//2]."""
    out = np.empty((B, SEQ, D), dtype=np.float32)
    for core in range(NCORES):
        b, j = divmod(core, G)
        co = np.asarray(core_outs[core]).astype(np.float32)
        for s in range(2):
            qb = SLOT_QB[s]
            blockT = co[s * ODC:(s + 1) * ODC, :]  # [256 dims, 512 tok]
            out[b, qb * QB:(qb + 1) * QB, ODC * j:ODC * (j + 1)] = blockT.T
        qb = SLOT_QB[2 + j // 2]
        d0 = (j % 2) * 512
        blockT = co[2 * ODC:4 * ODC, :]  # [512 dims, 512 tok]
        out[b, qb * QB:(qb + 1) * QB, d0:d0 + 512] = blockT.T
    return out


_CACHED = {}


def _get_nc(use_kpm=False):
    if use_kpm not in _CACHED:
        _CACHED[use_kpm] = build_program(use_kpm=use_kpm)
    return _CACHED[use_kpm]


def kernel(**inputs) -> np.ndarray:
    use_kpm = bool(np.asarray(inputs["key_padding_mask"]).any())
    nc = _get_nc(use_kpm)
    in_maps = [prep_core_inputs(inputs, core) for core in range(NCORES)]
    res = run_bass_kernel_spmd(nc, in_maps, core_ids=list(range(NCORES)))
    return assemble_output([res.results[c]["out"] for c in range(NCORES)])


if __name__ == "__main__":
    nc = build_program()
    print("program built ok")


# revision 59
# speedup vs baseline: 1.0173x; 1.0054x over previous
"""Trainium2 Bass kernel for nn_AttentionUnit (multi-head attention block), v3.

Reference math (B=2, S=2048, D=1024, H=16 heads, d_head=64, fp32):
    Q = q @ wq.T + bq ; K = k @ wk.T + bk ; V = v @ wv.T + bv
    S = QK^T / 8  (per head), causal mask + key-padding mask
    out = softmax(S) @ V  -> concat heads -> @ wo.T + bo
Sharding (8 cores): data-parallel over batch (2 groups of 4 cores),
tensor-parallel over heads (4 heads/core).  Column-parallel QKV,
row-parallel wo.

v3 changes vs v2 (156.0us):
  - Out-proj computed TRANSPOSED (out^T[dim, tok] psum tiles): the bias
    becomes a per-partition scalar so eviction is a 192ns tensor_scalar
    instead of a 658ns tensor_tensor (-15us DVE), and the partial/RS/
    output layout is [dims, tokens] (host re-transposes).
  - One ReduceScatter per q block, writing the external output directly
    (no rs_out bounce).  Block completion order is re-staged
    (qb0 -> [qb1 h0/h1] -> qb3 -> qb2 -> [qb1 h2/h3]) so the first
    three collectives drain the queue early and the tail collective
    carries a single 256KB-out block.
  - Exp split between the Act engine and the DVE by a tunable pattern;
    Act engine carries no DMAs anymore.
  - kq/v input DMAs front-loaded; score emission per-mt-gated as before.
"""

import os
import sys
from collections import deque
from contextlib import ExitStack

import numpy as np

try:
    import concourse.bass as bass
except ImportError:  # harness containers keep the repo at /opt/trn_rl_repo
    for _p in ("/opt/trn_rl_repo", "/root/.axon_site/_ro/trn_rl_repo"):
        if os.path.isdir(_p) and _p not in sys.path:
            sys.path.insert(0, _p)
    import concourse.bass as bass

from concourse import bacc

import ml_dtypes
import concourse.mybir as mybir
import concourse.tile as tile
from concourse.bass_utils import run_bass_kernel_spmd

BF16 = ml_dtypes.bfloat16

B = 2
SEQ = 2048
D = 1024
H = 16
DH = 64
NCORES = 8
G = 4            # tensor-parallel group size (cores per batch)
HPC = H // G     # heads per core
DPC = HPC * DH   # head dims per core (256)
QB = 512         # q block width
KT = 128         # k tile height
NMT = DPC // 128  # mt tiles of per-core head dims (2)
NDT = D // 128    # contraction tiles of the model dim (8)
NQB = SEQ // QB   # q blocks (4)
SUB = QB // KT    # k tiles per q block on the diagonal (4)
NOT = D // 128    # out^T dim tiles (8)
ODC = D // G      # out dims per core after RS (256)


def build_program(use_kpm=False):
    """Emit the SPMD program (identical on all 8 cores)."""
    fp32 = mybir.dt.float32
    bf16 = mybir.dt.bfloat16

    nc = bacc.Bacc(num_devices=NCORES)

    xqT = nc.declare_dram_parameter("xqT", [D, SEQ], bf16, False)
    xkT = nc.declare_dram_parameter("xkT", [D, SEQ], bf16, False)
    xvT = nc.declare_dram_parameter("xvT", [D, SEQ], bf16, False)
    wqT = nc.declare_dram_parameter("wqT", [D, DPC], bf16, False)
    wkT = nc.declare_dram_parameter("wkT", [D, DPC], bf16, False)
    wvT = nc.declare_dram_parameter("wvT", [D, DPC], bf16, False)
    wvb = nc.declare_dram_parameter("wvb", [1, DPC], bf16, False)
    woT = nc.declare_dram_parameter("woT", [DPC, D], bf16, False)
    bq2_d = nc.declare_dram_parameter("bq2", [128, NMT], fp32, False)
    bk2_d = nc.declare_dram_parameter("bk2", [128, NMT], fp32, False)
    tri_d = nc.declare_dram_parameter("tri", [KT, KT], bf16, False)
    ident_d = nc.declare_dram_parameter("ident", [128, 128], bf16, False)
    bo2_d = nc.declare_dram_parameter("bo2", [128, NOT], fp32, False)
    kpm_d = (nc.declare_dram_parameter("kpmT", [128, SEQ // 128], fp32, False)
             if use_kpm else None)
    # out^T layout: rows = [qb-block][dims-slice owned by this core],
    # cols = the block's 512 tokens.
    out_ext = nc.declare_dram_parameter("out", [NQB * ODC, QB], bf16,
                                        isOutput=True)

    partial_dram = nc.dram_tensor("partial", [NQB * D, QB], bf16)
    rs_out = nc.dram_tensor("rs_out", [NQB * ODC, QB], bf16)

    groups = [[0, 1, 2, 3], [4, 5, 6, 7]]

    with ExitStack() as ctx:
        tc = ctx.enter_context(tile.TileContext(nc, num_cores=NCORES))

        xpool = ctx.enter_context(tc.tile_pool(name="xp", bufs=28))
        persist = ctx.enter_context(tc.tile_pool(name="persist", bufs=1))
        ppool = ctx.enter_context(tc.tile_pool(name="pp", bufs=34))
        cqpool = ctx.enter_context(tc.tile_pool(name="cq", bufs=4))
        opool = ctx.enter_context(tc.tile_pool(name="op", bufs=4))
        spool = ctx.enter_context(tc.tile_pool(name="sp", bufs=8))
        psP = ctx.enter_context(tc.tile_pool(name="psP", bufs=2, space="PSUM"))
        psM = ctx.enter_context(tc.tile_pool(name="psM", bufs=2, space="PSUM"))
        psC = ctx.enter_context(tc.tile_pool(name="psC", bufs=2, space="PSUM"))

        # ---- small constants.  The ones needed in the first ~10us (exp
        # masks, K/Q biases) ride the otherwise-idle Act queue; the rest
        # (ident/bo2/wvb/wo, first used ~18us+) are DMAed later on the
        # gpsimd queue (see the "c2" sched unit) so they don't delay the
        # xq0/wq0 feed that gates the first exp. ----
        bq2_sb = persist.tile([128, NMT], fp32, tag="bq2")
        nc.scalar.dma_start(out=bq2_sb, in_=bq2_d[:, :])
        tri_sb = persist.tile([KT, KT], bf16, tag="tri")
        nc.scalar.dma_start(out=tri_sb, in_=tri_d[:, :])
        bk2_sb = persist.tile([128, NMT], fp32, tag="bk2")
        nc.gpsimd.dma_start(out=bk2_sb, in_=bk2_d[:, :])
        ones1 = persist.tile([1, 128], bf16, tag="ones1")
        nc.vector.memset(ones1, 1.0)
        # warm the Act function table (LoadActFuncSet ~1.3us) before the
        # first real exp needs it
        warm = persist.tile([1, 1], fp32, tag="warm")
        nc.vector.memset(warm, 0.0)
        nc.scalar.activation(out=warm, in_=warm,
                             func=mybir.ActivationFunctionType.Exp)
        kpm_sb = None
        if use_kpm:
            kpm_sb = persist.tile([128, SEQ // 128], fp32, tag="kpm")
            nc.scalar.dma_start(out=kpm_sb, in_=kpm_d[:, :])
        ident_sb = persist.tile([128, 128], bf16, tag="ident")
        bo2_sb = persist.tile([128, NOT], fp32, tag="bo2")
        wvb_sb = persist.tile([1, DPC], bf16, tag="wvb")
        wo_sb = [persist.tile([128, D], bf16, tag=f"wo{t}", name=f"wo{t}")
                 for t in range(NMT)]

        def consts2():
            nc.gpsimd.dma_start(out=ident_sb, in_=ident_d[:, :])
            nc.gpsimd.dma_start(out=bo2_sb, in_=bo2_d[:, :])
            nc.gpsimd.dma_start(out=wvb_sb, in_=wvb[0:1, :])
            for t in range(NMT):
                nc.gpsimd.dma_start(out=wo_sb[t],
                                    in_=woT[t * 128:(t + 1) * 128, :])

        # ---- persistent weights (consumed every block; load once) ----
        wk_t = [persist.tile([128, DPC], bf16, tag=f"wk{k}", name=f"wk{k}")
                for k in range(NDT)]
        wq_t = [persist.tile([128, DPC], bf16, tag=f"wq{k}", name=f"wq{k}")
                for k in range(NDT)]
        wv_t = [persist.tile([128, DPC], bf16, tag=f"wv{k}", name=f"wv{k}")
                for k in range(NDT)]

        # ---- persistent activations ----
        # K2/Q2 are mt-major: rows = the 128 head dims of heads (2mt, 2mt+1).
        K2 = [persist.tile([128, SEQ], bf16, tag=f"K2{t}", name=f"K2{t}")
              for t in range(NMT)]
        Q2 = [persist.tile([128, SEQ], bf16, tag=f"Q2{t}", name=f"Q2{t}")
              for t in range(NMT)]
        V_sb = [persist.tile([128, HPC, 65], bf16, tag=f"V{m}", name=f"V{m}")
                for m in range(SEQ // KT)]
        ctxT = [persist.tile([128, SEQ], bf16, tag=f"ctxT{t}", name=f"ctxT{t}")
                for t in range(NMT)]

        def kq_dma(blk):
            """Issue K/Q input DMAs for q/k columns [blk*QB, (blk+1)*QB).

            Block 0 gates the first exp, so its 32 transfers are spread
            over four queues (DVE/Act are idle at t=0) to land in ~4us
            instead of 8; later blocks ride sync/gpsimd as usual."""
            c0 = blk * QB
            xk_t, xq_t = [], []
            if blk == 0:
                e3 = [nc.sync, nc.gpsimd, nc.scalar]
                for k in range(NDT):
                    eng = e3[k % 3]
                    xk = xpool.tile([128, QB], bf16, tag="xt", name=f"xk0_{k}")
                    eng.dma_start(out=xk, in_=xkT[k * 128:(k + 1) * 128, c0:c0 + QB])
                    eng.dma_start(out=wk_t[k], in_=wkT[k * 128:(k + 1) * 128, :])
                    xq = xpool.tile([128, QB], bf16, tag="xt", name=f"xq0_{k}")
                    eng.dma_start(out=xq, in_=xqT[k * 128:(k + 1) * 128, c0:c0 + QB])
                    eng.dma_start(out=wq_t[k], in_=wqT[k * 128:(k + 1) * 128, :])
                    xk_t.append(xk)
                    xq_t.append(xq)
                return xk_t, xq_t
            eng = nc.gpsimd if blk == 1 else nc.sync
            for k in range(NDT):
                xk = xpool.tile([128, QB], bf16, tag="xt", name=f"xk{blk}_{k}")
                nc.sync.dma_start(out=xk, in_=xkT[k * 128:(k + 1) * 128, c0:c0 + QB])
                xq = xpool.tile([128, QB], bf16, tag="xt", name=f"xq{blk}_{k}")
                eng.dma_start(out=xq, in_=xqT[k * 128:(k + 1) * 128, c0:c0 + QB])
                xk_t.append(xk)
                xq_t.append(xq)
            return xk_t, xq_t

        def kq_gen(blk, mt, xk_t, xq_t):
            """K and Q projection matmuls for one mt half (yield/matmul)."""
            c0 = blk * QB
            pskq = psP.tile([128, 2 * QB], fp32, tag="p2", name="pskq")
            psk, psq = pskq[:, 0:QB], pskq[:, QB:2 * QB]
            for k in range(NDT):
                st, sp = (k == 0), (k == NDT - 1)
                nc.tensor.matmul(out=psk, rhs=xk_t[k],
                                 lhsT=wk_t[k][:, mt * 128:(mt + 1) * 128],
                                 start=st, stop=sp)
                yield
                nc.tensor.matmul(out=psq, rhs=xq_t[k],
                                 lhsT=wq_t[k][:, mt * 128:(mt + 1) * 128],
                                 start=st, stop=sp)
                yield
            nc.vector.tensor_scalar(
                out=K2[mt][:, c0:c0 + QB], in0=psk,
                scalar1=bk2_sb[:, mt:mt + 1], scalar2=None,
                op0=mybir.AluOpType.add)
            if blk == 0:
                # Act is idle before the first exp; evicting Q there
                # overlaps the serial DVE eviction pair at startup
                nc.scalar.activation(
                    out=Q2[mt][:, c0:c0 + QB], in_=psq,
                    bias=bq2_sb[:, mt:mt + 1],
                    func=mybir.ActivationFunctionType.Identity)
            else:
                nc.vector.tensor_scalar(
                    out=Q2[mt][:, c0:c0 + QB], in0=psq,
                    scalar1=bq2_sb[:, mt:mt + 1], scalar2=None,
                    op0=mybir.AluOpType.add)

        def v_dma(blk):
            c0 = blk * QB
            eng = nc.gpsimd if blk <= 1 else nc.sync
            xv_t = []
            for k in range(NDT):
                xv = xpool.tile([128, QB], bf16, tag="xt", name=f"xv{blk}_{k}")
                eng.dma_start(out=xv, in_=xvT[k * 128:(k + 1) * 128, c0:c0 + QB])
                if blk == 0:
                    eng.dma_start(out=wv_t[k], in_=wvT[k * 128:(k + 1) * 128, :])
                xv_t.append(xv)
            return xv_t

        def v_gen(blk, w, xv_t):
            """V projection half-block: tokens [blk*QB + w*256, +256) ->
            V_sb[4blk+2w], V_sb[4blk+2w+1].  Accumulates in psM quarter
            tiles (sequentially, short holds) so score pairs keep both
            psP banks for depth-2 exp during the long sc phases."""
            psv = psP.tile([128, 2 * QB], fp32, tag="p2", name="psv")
            ps = [psv[:, 0:DPC], psv[:, QB:QB + DPC]]
            for k in range(NDT):
                for i in range(2):
                    m2 = 2 * w + i
                    nc.tensor.matmul(out=ps[i], rhs=wv_t[k],
                                     lhsT=xv_t[k][:, m2 * 128:(m2 + 1) * 128],
                                     start=(k == 0), stop=False)
                    yield
            for i in range(2):
                mt = SUB * blk + 2 * w + i
                nc.tensor.matmul(out=ps[i], rhs=wvb_sb[0:1, :],
                                 lhsT=ones1[0:1, :], start=False, stop=True)
                nc.vector.tensor_copy(
                    out=V_sb[mt][:, :, 0:64],
                    in_=ps[i].rearrange("p (h e) -> p h e", h=HPC))
                nc.vector.memset(V_sb[mt][:, :, 64:65], 1.0)
                yield

        def sc_head(qb, h, pts, pump, reserve):
            """Scores + exp + causal mask for one head of q block qb."""
            q0 = qb * QB
            mt, hh = divmod(h, 2)
            krows = slice(64 * hh, 64 * hh + 64)
            nfull = SUB * qb
            for i in range(nfull // 2):
                # two full k tiles share a 2-bank PSUM tile and one wide exp
                k0 = 2 * i
                reserve("pt2")
                s2 = psP.tile([128, 2 * QB], fp32, tag="p2", name=f"s2{h}_{i}")
                pt2 = ppool.tile([128, 2 * QB], bf16, tag="pt2",
                                 name=f"pt2{h}_{i}")
                for d in range(2):
                    nc.tensor.matmul(
                        out=s2[:, d * QB:(d + 1) * QB],
                        lhsT=K2[mt][krows, (k0 + d) * KT:(k0 + d + 1) * KT],
                        rhs=Q2[mt][krows, q0:q0 + QB],
                        start=True, stop=True)
                    pump(1)
                nc.scalar.activation(
                    out=pt2, in_=s2,
                    func=mybir.ActivationFunctionType.Exp)
                if use_kpm:
                    for d in range(2):
                        nc.vector.tensor_scalar(
                            out=pt2[:, d * QB:(d + 1) * QB],
                            in0=pt2[:, d * QB:(d + 1) * QB],
                            scalar1=kpm_sb[:, k0 + d:k0 + d + 1], scalar2=None,
                            op0=mybir.AluOpType.mult)
                pts[h, k0] = pt2[:, 0:QB]
                pts[h, k0 + 1] = pt2[:, QB:2 * QB]
                pump(2)
            for kti in range(nfull, nfull + SUB):
                o = 128 * (kti - nfull)
                reserve("pt")
                s_ps = psM.tile([128, QB], fp32, tag="m", name=f"s{h}_{kti}")
                nc.tensor.matmul(
                    out=s_ps[:, o:QB],
                    lhsT=K2[mt][krows, kti * KT:(kti + 1) * KT],
                    rhs=Q2[mt][krows, q0 + o:q0 + QB],
                    start=True, stop=True)
                pt = ppool.tile([128, QB], bf16, tag="pt", name=f"pt{h}_{kti}")
                nc.scalar.activation(
                    out=pt[:, o:QB], in_=s_ps[:, o:QB],
                    func=mybir.ActivationFunctionType.Exp)
                nc.vector.tensor_mul(
                    out=pt[:, o:o + KT], in0=pt[:, o:o + KT], in1=tri_sb)
                if use_kpm:
                    nc.vector.tensor_scalar(
                        out=pt[:, o:QB], in0=pt[:, o:QB],
                        scalar1=kpm_sb[:, kti:kti + 1], scalar2=None,
                        op0=mybir.AluOpType.mult)
                pts[h, kti] = pt
                pump(3)

        def pv_gen(qb, p, pts):
            """PV for head pair p; ctx lands q-major; one transpose/chunk."""
            q0 = qb * QB
            h0, h1 = 2 * p, 2 * p + 1

            def flush(qs, cq):
                tr_ps = psC.tile([128, 128], bf16, tag="c")
                nc.tensor.transpose(out=tr_ps, in_=cq, identity=ident_sb)
                nc.vector.tensor_copy(
                    out=ctxT[p][:, q0 + qs * 128:q0 + (qs + 1) * 128],
                    in_=tr_ps)

            pend = None
            for qs in range(SUB):
                cq = cqpool.tile([128, 128], bf16, tag="cq")
                for j, h in enumerate((h0, h1)):
                    ctx_ps = psC.tile([128, 65], fp32, tag="c", name=f"ctx{j}")
                    for kti in range(SUB * qb + qs + 1):
                        nc.tensor.matmul(
                            out=ctx_ps,
                            lhsT=pts[h, kti][:, qs * 128:(qs + 1) * 128],
                            rhs=V_sb[kti][:, h, :],
                            start=(kti == 0), stop=(kti == SUB * qb + qs))
                    rcp = spool.tile([128, 1], fp32, tag="rcp")
                    nc.vector.reciprocal(out=rcp, in_=ctx_ps[:, 64:65])
                    nc.vector.tensor_scalar(
                        out=cq[:, 64 * j:64 * j + 64], in0=ctx_ps[:, 0:64],
                        scalar1=rcp, scalar2=None, op0=mybir.AluOpType.mult)
                    yield
                if pend is not None:
                    flush(*pend)
                pend = (qs, cq)
            flush(*pend)

        # partial_dram block slots in completion order (qb0, qb1, qb3,
        # qb2); each block fires its own RS as soon as its partials land.
        # assemble_output inverts PSLOT.
        PSLOT = {0: 0, 1: 1, 3: 2, 2: 3}

        def op_gen(qb):
            """Transposed out-proj for block qb -> partial -> ReduceScatter.

            psum tiles are out^T[dt*128:(dt+1)*128, 512 tokens]; the bias is
            per-partition so eviction is one tensor_scalar.  The RS scatters
            a partial slot's D rows over the 4-core group; core j receives
            dims [256j, 256j+256) directly into the external output.
            """
            q0 = qb * QB
            p0 = PSLOT[qb] * D
            for dt in range(NOT):
                ps = psM.tile([128, QB], fp32, tag="m", name="pso")
                for t in range(NMT):
                    nc.tensor.matmul(
                        out=ps,
                        lhsT=wo_sb[t][:, dt * 128:(dt + 1) * 128],
                        rhs=ctxT[t][:, q0:q0 + QB],
                        start=(t == 0), stop=(t == NMT - 1))
                    yield
                po = opool.tile([128, QB], bf16, tag="po")
                # the last block runs after all exps, so the Act engine is
                # free to take half its evictions + partial writes and
                # shorten the tail-exposed chain
                if qb == 2 and dt % 2:
                    nc.scalar.activation(
                        out=po, in_=ps, bias=bo2_sb[:, dt:dt + 1],
                        func=mybir.ActivationFunctionType.Identity)
                    nc.scalar.dma_start(
                        out=partial_dram[p0 + dt * 128:p0 + (dt + 1) * 128, :],
                        in_=po)
                else:
                    nc.vector.tensor_scalar(
                        out=po, in0=ps, scalar1=bo2_sb[:, dt:dt + 1],
                        scalar2=None, op0=mybir.AluOpType.add)
                    # partials never share the Pool queue with the RS's (a
                    # collective blocks its queue until it completes)
                    nc.sync.dma_start(
                        out=partial_dram[p0 + dt * 128:p0 + (dt + 1) * 128, :],
                        in_=po)
            s = PSLOT[qb]
            if s == 2:
                return  # slot 2 (qb3) is carried by the merged tail RS
            if s == 3:
                # qb3+qb2 finish nearly together at the exp-paced tail; one
                # merged RS beats two serialized 15us fixed costs
                nc.gpsimd.collective_compute(
                    "ReduceScatter",
                    mybir.AluOpType.add,
                    replica_groups=groups,
                    ins=[partial_dram[2 * D:4 * D, :]],
                    outs=[rs_out[2 * ODC:4 * ODC, :]],
                )
            else:
                nc.gpsimd.collective_compute(
                    "ReduceScatter",
                    mybir.AluOpType.add,
                    replica_groups=groups,
                    ins=[partial_dram[s * D:(s + 1) * D, :]],
                    outs=[rs_out[s * ODC:(s + 1) * ODC, :]],
                )

        # Emission schedule.  sc units emit inline (exp-paced, pumping
        # queued exp-free work between tiles); everything else is queued
        # and drained as filler.  Block completion order:
        # qb0 -> qb1(h0,h1) -> qb3 -> qb2 -> qb1(h2,h3), so RS[0], RS[3]
        # drain the collective queue early and the tail RS merges the two
        # late blocks.  kq projection halves (kqg blk mt) are queued
        # separately so the mt0 halves of blocks 2/3 (which gate the long
        # qb3 exp phase) drain before lower-priority filler.
        # xpool recycles 28 'xt' slots; kq/v gens must drain in DMA
        # emission order (each block's BOTH mt halves before the block
        # two later is touched) or the PE FIFO deadlocks on slot reuse.
        # Exp phases: qb0, qb1, qb2(h01) cover the PE with projection /
        # pv / op filler until all kq blocks are projected, then qb3's
        # long phase runs, then qb2(h23) closes.  Completion order is
        # qb0, qb1, qb3, qb2 with one RS each.
        sched = [
            ("kq", 0), ("c2",), ("kqg", 0, 0), ("kqg", 0, 1),
            ("sc", 0, 0), ("sc", 0, 1),
            ("kq", 1), ("kqg", 1, 0), ("kqg", 1, 1),
            ("v", 0, 0), ("v", 0, 1),
            ("sc", 0, 2), ("sc", 0, 3),
            ("kq", 2), ("kqg", 2, 0), ("kqg", 2, 1),
            ("sc", 1, 0), ("sc", 1, 1),
            ("pv", 0, 0), ("pv", 0, 1),
            ("op", 0),
            ("sc", 1, 2), ("sc", 1, 3),
            ("kq", 3),
            ("v", 1, 0), ("v", 1, 1),
            ("pv", 1, 0), ("pv", 1, 1),
            ("op", 1),
            ("kqg", 3, 0), ("kqg", 3, 1),
            ("v", 2, 0), ("v", 2, 1), ("v", 3, 0), ("v", 3, 1),
            ("sc", 2, 0), ("sc", 2, 1),
            ("pv", 2, 0),
            ("sc", 3, 0), ("sc", 3, 1), ("sc", 3, 2), ("sc", 3, 3),
            ("pv", 3, 0), ("pv", 3, 1),
            ("op", 3),
            ("sc", 2, 2), ("sc", 2, 3),
            ("pv", 2, 1),
            ("op", 2),
        ]
        pts_all = {qb: {} for qb in range(NQB)}
        queue = deque()
        kq_tiles = {}
        v_drained = set()  # (blk, w) halves fully emitted
        # live pts-tile counters: a new exp's pool slot is freed by pv
        # matmul reads of the old occupant, which must already be emitted
        # in the PE FIFO or the slot-WAR closes a dependency cycle.
        live = {"pt2": 0, "pt": 0}
        CAP = {"pt2": 31, "pt": 31}

        def v_ready(qb):
            """pv(qb, *) may only emit once v halves for kti<=4qb+3 are."""
            return all((b, w) in v_drained
                       for b in range(qb + 1) for w in range(2))

        def pump(n, light=False):
            """Drain n generator steps.  light=True prefers pv/op units
            (they hold no psP bank, so score pairs keep exp depth 2),
            scanning past queued kq/v units but stopping at the first pv
            whose V halves are not yet emitted (emission order is
            dependency order for the Tile tracker)."""
            while n > 0 and queue:
                idx = 0
                if light:
                    while idx < len(queue) and queue[idx][0][0] in ("kq", "v"):
                        idx += 1
                    if idx >= len(queue):
                        return
                    key = queue[idx][0]
                    if key[0] == "pv" and not v_ready(key[1]):
                        return
                key = queue[idx][0]
                try:
                    next(queue[idx][1])
                    n -= 1
                except StopIteration:
                    if key[0] == "v":
                        v_drained.add((key[1], key[2]))
                    elif key[0] == "pv":
                        qb = key[1]
                        live["pt2"] -= 2 * ((SUB * qb) // 2)
                        live["pt"] -= 2 * SUB
                    del queue[idx]

        def reserve(tag):
            """Before allocating a pts tile: force FIFO drains until the
            pool has a safe slot, so slot-WAR readers are always already
            in the PE FIFO."""
            while live[tag] >= CAP[tag] and queue:
                pump(4)
            live[tag] += 1

        def drain_sel(pred):
            """Drain kq units in queue order through the last one matching
            pred (slot-reuse safety: kq gens consume xt tiles in emission
            order), leaving non-kq units queued."""
            if not any(pred(key) for key, _ in queue):
                return
            last = max(i for i, (key, _) in enumerate(queue) if pred(key))
            kept = []
            for _ in range(last + 1):
                key, gen = queue.popleft()
                if key[0] == "kq":
                    for _ in gen:
                        pass
                else:
                    kept.append((key, gen))
            for item in reversed(kept):
                queue.appendleft(item)

        for unit in sched:
            kind = unit[0]
            if kind == "c2":
                consts2()
            elif kind == "kq":
                kq_tiles[unit[1]] = kq_dma(unit[1])
            elif kind == "kqg":
                blk, mt = unit[1], unit[2]
                xk_t, xq_t = kq_tiles[blk]
                queue.append((("kq", blk, mt), kq_gen(blk, mt, xk_t, xq_t)))
            elif kind == "v":
                blk, w = unit[1], unit[2]
                if w == 0:
                    kq_tiles["v", blk] = v_dma(blk)
                queue.append((unit, v_gen(blk, w, kq_tiles["v", blk])))
            elif kind == "pv":
                queue.append((unit, pv_gen(unit[1], unit[2], pts_all[unit[1]])))
            elif kind == "op":
                queue.append((unit, op_gen(unit[1])))
            else:
                qb, h = unit[1], unit[2]
                # writers this head reads must be fully emitted first
                drain_sel(lambda key: key[0] == "kq" and key[1] <= qb
                          and key[2] <= h // 2)
                sc_head(qb, h, pts_all[qb], pump, reserve)
        while queue:
            pump(1000)

        # rs_out -> out_ext bounces (collectives cannot write IO
        # tensors), emitted last so their RS-completion waits never
        # head-of-line-block earlier queue work.  Slots 0-2 are long done
        # when these run; the tail slot is split over two queues.
        for s in range(NQB - 1):
            osb = opool.tile([128, 2 * QB], bf16, tag="osb")
            nc.sync.dma_start(
                out=osb.rearrange("p (t q) -> p t q", t=2),
                in_=rs_out[s * ODC:(s + 1) * ODC, :].rearrange(
                    "(t p) q -> p t q", p=128))
            nc.sync.dma_start(
                out=out_ext[s * ODC:(s + 1) * ODC, :].rearrange(
                    "(t p) q -> p t q", p=128),
                in_=osb.rearrange("p (t q) -> p t q", t=2))
        s = NQB - 1
        for i, eng in enumerate((nc.sync, nc.scalar)):
            rows = slice(s * ODC + i * 128, s * ODC + (i + 1) * 128)
            osb = opool.tile([128, QB], bf16, tag="osb2")
            eng.dma_start(out=osb, in_=rs_out[rows, :])
            eng.dma_start(out=out_ext[rows, :], in_=osb)

    nc.compile()
    return nc


def prep_core_inputs(inputs, core):
    """Host-side shard/layout prep for one core.  Pure layout + dtype work."""
    b, g = divmod(core, G)
    sl = slice(g * DPC, (g + 1) * DPC)
    s = 1.0 / np.sqrt(DH)

    def xT(x):
        return np.ascontiguousarray(np.asarray(x)[b].T).astype(BF16)

    def wT(w, scale=1.0):
        wt = np.asarray(w)[sl, :].T.astype(np.float32)
        return (wt * scale).astype(BF16)

    def b2(bias, scale=1.0):
        bb = (np.asarray(bias)[sl].astype(np.float32) * scale)
        return np.ascontiguousarray(bb.reshape(NMT, 128).T)

    kpm = np.asarray(inputs["key_padding_mask"])
    tri = (np.arange(KT)[:, None] <= np.arange(KT)[None, :]).astype(BF16)

    d = {
        "xqT": xT(inputs["q_input"]),
        "xkT": xT(inputs["k_input"]),
        "xvT": xT(inputs["v_input"]),
        "wqT": wT(inputs["wq"], s),
        "wkT": wT(inputs["wk"]),
        "wvT": wT(inputs["wv"]),
        "wvb": np.asarray(inputs["bv"])[sl].astype(BF16)[None, :],
        "woT": np.ascontiguousarray(np.asarray(inputs["wo"]).T[sl, :]).astype(BF16),
        "bq2": b2(inputs["bq"], s),
        "bk2": b2(inputs["bk"]),
        "tri": tri,
        "ident": np.eye(128, dtype=BF16),
        # out^T bias: per-partition scalars, one column per 128-dim tile
        "bo2": np.ascontiguousarray(
            np.asarray(inputs["bo"]).astype(np.float32).reshape(NOT, 128).T / G),
    }
    if kpm.any():
        # kpmT[p, kti] = 0.0 for padded key (128*kti + p), else 1.0
        d["kpmT"] = np.ascontiguousarray(
            1.0 - kpm[b].astype(np.float32).reshape(SEQ // 128, 128).T)
    return d


SLOT_QB = [0, 1, 3, 2]  # partial/output slot -> q block (inverse of PSLOT)


def assemble_output(core_outs):
    """core 4b+j returns out^T chunks.  Slots 0/1 come from single-block
    RS's: rows [s*256, s*256+256) hold dims [256j, 256j+256) of block
    SLOT_QB[s].  Rows [512:1024) come from the merged slot-2+3 RS, which
    scatters 512 contiguous rows per core: core j holds dims
    [(j%2)*512, ..+512) of block SLOT_QB[2 + j//2]."""
    out = np.empty((B, SEQ, D), dtype=np.float32)
    for core in range(NCORES):
        b, j = divmod(core, G)
        co = np.asarray(core_outs[core]).astype(np.float32)
        for s in range(2):
            qb = SLOT_QB[s]
            blockT = co[s * ODC:(s + 1) * ODC, :]  # [256 dims, 512 tok]
            out[b, qb * QB:(qb + 1) * QB, ODC * j:ODC * (j + 1)] = blockT.T
        qb = SLOT_QB[2 + j // 2]
        d0 = (j % 2) * 512
        blockT = co[2 * ODC:4 * ODC, :]  # [512 dims, 512 tok]
        out[b, qb * QB:(qb + 1) * QB, d0:d0 + 512] = blockT.T
    return out


_CACHED = {}


def _get_nc(use_kpm=False):
    if use_kpm not in _CACHED:
        _CACHED[use_kpm] = build_program(use_kpm=use_kpm)
    return _CACHED[use_kpm]


def kernel(**inputs) -> np.ndarray:
    use_kpm = bool(np.asarray(inputs["key_padding_mask"]).any())
    nc = _get_nc(use_kpm)
    in_maps = [prep_core_inputs(inputs, core) for core in range(NCORES)]
    res = run_bass_kernel_spmd(nc, in_maps, core_ids=list(range(NCORES)))
    return assemble_output([res.results[c]["out"] for c in range(NCORES)])


if __name__ == "__main__":
    nc = build_program()
    print("program built ok")
